# revision 1
# baseline (speedup 1.0000x reference)
"""CensusLoss Trainium2 kernel.

Census transform loss: grayscale -> 48 shifted binary comparisons (7x7 patch,
reflect pad 3) -> mean |pred_census - target_census|.

Sharding: pure data parallel, batch dim B=8 across 8 NeuronCores (one image
per core). Each core emits exact integer partial sums (in f32); the host
combines them and divides.

Per-core pipeline:
  1. gray = 0.299R + 0.587G + 0.114B (ACT muls -> bf16, DVE adds), written
     column-reflect-padded DIRECTLY into the center rows of the "band" tile
     (row width 520 keeps every bf16 row 4B-aligned => DVE 2x_1P mode).
  2. band layout: partition p holds padded rows 4p..4p+9 flattened
     ([128, 5200]); only the 3+3 halo rows need DMAs (partition-shifted
     SBUF->SBUF affine copies from the neighbors' center rows), plus per-row
     reflect copies at the image edges. bandB = bandA shifted one element
     (keeps odd-column-offset neighbor reads 4B-aligned).
  3. Per offset (di,dj): cmpP = is_gt(center, neighbor), cmpT likewise
     (bf16 2x mode, ~1us per [128,2048] op). Every 6th offset instead
     computes d = center - neighbor on the otherwise-idle GPSIMD engine and
     binarizes on DVE with tensor_scalar(d > 0) in 4x mode (bf16 subtraction
     sign is exact, so results are identical).
     sum(xor) = sum(cmpP) + sum(cmpT) - 2*sum(cmpP*cmpT):
       - sum(cmpP): ACT activation(Copy) with accum_out (idle engine)
       - sum(cmpT): PE ones-matmul accumulated in PSUM
       - sum(cmpP*cmpT): PE gram blocks accumulated in PSUM; only the
         diagonal of the [128,128] result is meaningful.
  4. Host: total = sum(acc48) + sum(sums) - 2*trace(prod), exact integers.

Comparisons run in bf16: f32->bf16 rounding is monotonic, so only near-ties
can flip a comparison; measured effect on the mean is ~2e-6 relative.
"""

import numpy as np

B, C, H, W = 8, 3, 512, 512
N_CORES = 8
PAD = 3
N_OFF = 48
Wp = 520            # padded row width (518 used + 2 spare, even for alignment)
COL0 = 4            # padded col of gray col 0 (even => 4B-aligned in bf16)
RPP = 4             # gray rows per partition (512 / 128)
BAND_ROWS = RPP + 2 * PAD            # 10
BAND_LEN = BAND_ROWS * Wp            # 5200
ROW_TILE = RPP * Wp                  # 2080
FREE = RPP * W                       # 2048

_CACHE = {}


def _offsets():
    # even-dj offsets first: they only need the bandA construction, so the
    # main loop starts while the shifted bandB copies are still in flight
    evens, odds = [], []
    for di in range(-PAD, PAD + 1):
        for dj in range(-PAD, PAD + 1):
            if di == 0 and dj == 0:
                continue
            (evens if dj % 2 == 0 else odds).append((di, dj))
    return evens + odds


def _build_bass(n_off=N_OFF, repeat=1):
    from concourse import bacc, mybir
    from concourse.ap import AP
    from concourse.tile import TileContext
    from concourse.alu_op_type import AluOpType as op

    dt = mybir.dt
    # Bacc (not raw Bass): its compile() pass splits multi-sem waits into
    # event-semaphore NOPs — TRN2 instructions allow at most one wait each.
    nc = bacc.Bacc("TRN2", debug=False)

    pred = nc.dram_tensor("pred", [C, H, W], dt.float32, kind="ExternalInput")
    target = nc.dram_tensor("target", [C, H, W], dt.float32, kind="ExternalInput")
    acc48_out = nc.dram_tensor("acc48_out", [128, max(n_off, 1)], dt.float32,
                               kind="ExternalOutput")
    sums_out = nc.dram_tensor("sums_out", [1, 512], dt.float32,
                              kind="ExternalOutput")
    prod_out = nc.dram_tensor("prod_out", [128, 128], dt.float32,
                              kind="ExternalOutput")

    def band_view(t, r0, c0):
        # [128, RPP rows, W cols] view of a band tile at row r0, col c0
        return t.rearrange("p (r w) -> p r w", w=Wp)[
            :, r0:r0 + RPP, c0:c0 + W]

    with TileContext(nc) as tc:
      with tc.tile_pool(name="sbuf", bufs=1) as pool:
        for _rep in range(repeat):
            bands = {}
            for nm in ("p", "t"):
                for ab in ("A", "B"):
                    bands[nm + ab] = pool.tile(
                        [128, BAND_LEN], dt.bfloat16,
                        name=f"band_{nm}{ab}", tag=f"band_{nm}{ab}",
                    )

            # channel loads interleaved across the two HWDGE queues (SP +
            # ACT-seq) with pred's channels at the FRONT of both queues:
            # pred finishes first so its gray/band build overlaps target's
            # remaining input transfers
            chs = {}
            load_order = [("p", 0, nc.sync), ("p", 1, nc.scalar),
                          ("p", 2, nc.sync), ("t", 0, nc.scalar),
                          ("t", 1, nc.sync), ("t", 2, nc.scalar)]
            for nm, c, q in load_order:
                src = pred if nm == "p" else target
                cht = pool.tile([128, FREE], dt.float32,
                                name=f"ch_{nm}{c}", tag=f"ch_{nm}{c}", bufs=1)
                q.dma_start(
                    out=cht,
                    in_=src.ap()[c].rearrange("(p r) w -> p (r w)", p=128),
                )
                chs[(nm, c)] = cht

            for nm, src in (("p", pred), ("t", target)):
                qeng = nc.sync if nm == "p" else nc.scalar
                ch = [chs[(nm, c)] for c in range(3)]
                g1 = pool.tile([128, FREE], dt.bfloat16, name=f"g1_{nm}",
                               tag="g1", bufs=1)
                nc.scalar.mul(g1, ch[0], 0.299)
                gb = pool.tile([128, FREE], dt.bfloat16, name=f"gb_{nm}",
                               tag="gb", bufs=1)
                nc.scalar.mul(gb, ch[1], 0.587)
                gc = pool.tile([128, FREE], dt.bfloat16, name=f"gc_{nm}",
                               tag="gc", bufs=1)
                nc.scalar.mul(gc, ch[2], 0.114)
                g2 = pool.tile([128, FREE], dt.bfloat16, name=f"g2_{nm}",
                               tag="g2", bufs=1)
                nc.vector.tensor_add(g2, g1, gb)
                g3 = pool.tile([128, FREE], dt.bfloat16, name=f"g3_{nm}",
                               tag="g3", bufs=1)
                nc.vector.tensor_add(g3, g2, gc)

                g3v = g3.rearrange("p (r w) -> p r w", w=W)
                # gray rows are written straight into the band tile's center
                # slots (rows 3..6): bandA then only needs the halo DMAs
                bA = bands[nm + "A"]
                padv = bA.rearrange("p (r w) -> p r w", w=Wp)[:, PAD:PAD + RPP, :]
                # zero the 2 spare cols (0 and 519) so halo DMAs carry
                # defined bytes
                nc.vector.memset(
                    AP(bA.tensor, bA.offset + PAD * Wp,
                       [[BAND_LEN, 128], [Wp, RPP], [Wp - 1, 2]]),
                    0.0)
                # center cols: gray col w -> padded col w+COL0
                nc.vector.tensor_copy(out=padv[:, :, COL0:COL0 + W], in_=g3v)
                # reflect cols: padded col COL0-t = gray col t (t=1..3)
                nc.vector.tensor_copy(out=padv[:, :, 1:4], in_=g3v[:, :, 3:0:-1])
                # padded col COL0+W-1+t = gray col W-1-t
                nc.vector.tensor_copy(out=padv[:, :, 516:519],
                                      in_=g3v[:, :, 510:507:-1])

            # ---- halo construction, all SBUF->SBUF within the band ----
            # center slot s (gray row 4p+s) lives at band offset (3+s)*Wp
            for nm in ("t", "p"):
                qeng = nc.sync if nm == "p" else nc.scalar
                bA = bands[nm + "A"]
                pstride_b = bA.ap[0][0]
                # top halo: band[p][slots 0..2] <- band[p-1][center slots 1..3]
                qeng.dma_start(
                    out=AP(bA.tensor, bA.offset + 1 * pstride_b,
                           [[pstride_b, 127], [1, 3 * Wp]]),
                    in_=AP(bA.tensor, bA.offset + 4 * Wp,
                           [[pstride_b, 127], [1, 3 * Wp]]))
                # bottom halo: band[p][slots 7..9] <- band[p+1][center 0..2]
                qeng.dma_start(
                    out=AP(bA.tensor, bA.offset + 7 * Wp,
                           [[pstride_b, 127], [1, 3 * Wp]]),
                    in_=AP(bA.tensor, bA.offset + 1 * pstride_b + 3 * Wp,
                           [[pstride_b, 127], [1, 3 * Wp]]))
                # reflect edges: partition 0 top = gray rows 3,2,1 (center
                # slots 3,2,1); partition 127 bottom = gray rows 510,509,508
                # (center slots 2,1,0)
                for s_band, slot in ((0, 3), (1, 2), (2, 1)):
                    qeng.dma_start(
                        out=AP(bA.tensor, bA.offset + s_band * Wp,
                               [[pstride_b, 1], [1, Wp]]),
                        in_=AP(bA.tensor, bA.offset + (PAD + slot) * Wp,
                               [[pstride_b, 1], [1, Wp]]))
                for s_band, slot in ((7, 2), (8, 1), (9, 0)):
                    qeng.dma_start(
                        out=AP(bA.tensor,
                               bA.offset + 127 * pstride_b + s_band * Wp,
                               [[pstride_b, 1], [1, Wp]]),
                        in_=AP(bA.tensor,
                               bA.offset + 127 * pstride_b + (PAD + slot) * Wp,
                               [[pstride_b, 1], [1, Wp]]))
            # bandB = bandA shifted left one element (last element unused and
            # never read by any compute view)
            for nm in ("p", "t"):
                qeng = nc.sync if nm == "p" else nc.scalar
                bA, bB = bands[nm + "A"], bands[nm + "B"]
                qeng.dma_start(out=bB[:, 0:BAND_LEN - 1],
                               in_=bA[:, 1:BAND_LEN])

            # ---- main loop ----
            centers = {nm: band_view(bands[nm + "A"], PAD, COL0)
                       for nm in ("p", "t")}
            acc48 = pool.tile([128, max(n_off, 1)], dt.float32,
                              name="acc48", tag="acc48")
            nc.vector.memset(acc48, 0.0)
            ones = pool.tile([128, 1], dt.bfloat16, name="ones", tag="ones")
            nc.vector.memset(ones, 1.0)
            with tc.tile_pool(name="psum", bufs=1, space="PSUM") as ppool:
                prod = ppool.tile([128, 128], dt.float32, name="prod")
                sums = ppool.tile([1, 512], dt.float32, name="sums")
                offs = _offsets()[:n_off]
                # every 8th offset's cmpP sum goes to PE instead of ACT
                pe_sum_idx = {i for i in range(len(offs)) if i % 8 == 7}
                # a subset of offsets computes d = center - neighbor on the
                # (otherwise idle) GPSIMD engine, then binarizes on DVE with
                # tensor_scalar(is_gt, 0) in 4x mode — bf16 subtraction sign
                # is exact, so results are identical to a direct is_gt
                gp_n = int(_CACHE.get("gp_n", 8))
                gp_idx = {i for i in range(len(offs)) if i % 6 == 5}
                gp_idx = set(sorted(gp_idx)[:gp_n])
                for i, (di, dj) in enumerate(offs):
                    cmps = {}
                    for nm in ("p", "t"):
                        if dj % 2 == 0:
                            nb = band_view(bands[nm + "A"], PAD + di, COL0 + dj)
                        else:
                            nb = band_view(bands[nm + "B"], PAD + di,
                                           COL0 + dj - 1)
                        cmp = pool.tile([128, FREE], dt.bfloat16,
                                        name=f"cmp_{nm}_{i}", tag=f"cmp_{nm}",
                                        bufs=8)
                        if i in gp_idx:
                            dsub = pool.tile([128, FREE], dt.bfloat16,
                                             name=f"d_{nm}_{i}", tag=f"d_{nm}",
                                             bufs=2)
                            nc.gpsimd.tensor_tensor(
                                out=dsub.rearrange("p (r w) -> p r w", w=W),
                                in0=centers[nm], in1=nb, op=op.subtract)
                            nc.vector.tensor_scalar(
                                out=cmp, in0=dsub, scalar1=0.0, scalar2=None,
                                op0=op.is_gt)
                        else:
                            nc.vector.tensor_tensor(
                                out=cmp.rearrange("p (r w) -> p r w", w=W),
                                in0=centers[nm], in1=nb, op=op.is_gt)
                        cmps[nm] = cmp
                    if i in pe_sum_idx:
                        for c in range(FREE // 512):
                            nc.tensor.matmul(
                                sums[0:1, :], ones[:, 0:1],
                                cmps["p"][:, c * 512:(c + 1) * 512],
                                start=False, stop=False,
                                skip_group_check=True)
                    else:
                        dact = pool.tile([128, FREE], dt.bfloat16,
                                         name=f"dact_{i}", tag="dact", bufs=1)
                        nc.scalar.activation(
                            out=dact, in_=cmps["p"],
                            func=mybir.ActivationFunctionType.Copy,
                            accum_out=acc48[:, i:i + 1])
                    for c in range(FREE // 128):
                        nc.tensor.matmul(
                            prod[:, :],
                            cmps["p"][:, c * 128:(c + 1) * 128],
                            cmps["t"][:, c * 128:(c + 1) * 128],
                            start=(i == 0 and c == 0),
                            stop=(i == len(offs) - 1 and c == FREE // 128 - 1),
                            skip_group_check=True)
                    for c in range(FREE // 512):
                        nc.tensor.matmul(
                            sums[0:1, :], ones[:, 0:1],
                            cmps["t"][:, c * 512:(c + 1) * 512],
                            start=(i == 0 and c == 0),
                            stop=(i == len(offs) - 1 and c == FREE // 512 - 1),
                            skip_group_check=True)

                prod_sb = pool.tile([128, 128], dt.float32, name="prod_sb",
                                    tag="prod_sb")
                sums_sb = pool.tile([1, 512], dt.float32, name="sums_sb",
                                    tag="sums_sb")
                if n_off == 0:
                    nc.vector.memset(prod_sb, 0.0)
                    nc.vector.memset(sums_sb, 0.0)
                else:
                    nc.vector.tensor_copy(out=prod_sb, in_=prod)
                    nc.vector.tensor_copy(out=sums_sb, in_=sums)
                nc.sync.dma_start(out=acc48_out.ap(), in_=acc48)
                nc.sync.dma_start(out=prod_out.ap(), in_=prod_sb)
                nc.sync.dma_start(out=sums_out.ap(), in_=sums_sb)

    nc.finalize()
    return nc


def kernel(pred: np.ndarray, target: np.ndarray) -> np.ndarray:
    from concourse import bass_utils

    if "nc" not in _CACHE:
        _CACHE["nc"] = _build_bass()
    nc = _CACHE["nc"]

    pred = np.ascontiguousarray(pred, dtype=np.float32)
    target = np.ascontiguousarray(target, dtype=np.float32)
    in_maps = [
        {"pred": pred[b], "target": target[b]} for b in range(N_CORES)
    ]
    res = bass_utils.run_bass_kernel_spmd(nc, in_maps,
                                          core_ids=list(range(N_CORES)))
    total = 0.0
    for r in res.results:
        total += float(r["acc48_out"].astype(np.float64).sum())
        total += float(r["sums_out"].astype(np.float64).sum())
        total -= 2.0 * float(np.diag(r["prod_out"]).astype(np.float64).sum())
    mean = total / (B * N_OFF * H * W)
    return np.array(mean, dtype=np.float32)



# revision 3
# speedup vs baseline: 1.5129x; 1.5129x over previous
"""CensusLoss Trainium2 kernel (24-offset antipodal scheme).

Census transform loss: grayscale -> 48 shifted binary comparisons (7x7 patch,
reflect pad 3) -> mean |pred_census - target_census|.

Key identity: for offset d and its antipode -d, the per-pixel XOR counts are
equal except at bf16 ties and reflect-pad borders (measured ~1e-5 relative on
these inputs), so total48 ~= 2 * total24 over the 24 offsets with di>0 or
(di==0 and dj>0).  Per offset d:
  contrib_d = sum(cmpP) + sum(cmpT) - 2*sum(cmpP*cmpT)
Some P-side planes are produced as SIGNS s = sign(dP) in {-1,0,1} (GPSIMD
subtract + ACT Sign) instead of {0,1} compares; for those offsets
  contrib_d = (sum(sP) + N)/2 - sum(sP*cmpT)
(the T-side plane sums cancel exactly; ties only add ~1e-4 noise).

All cross sums ride the PE: per 128-column block, a gram matmul accumulates
sum(f*g) on the PSUM diagonal, and 1-column matmuls against a ones vector
accumulate the plane sums (matmul cost is output-free-size, so these are
almost free).

Sharding: pure data parallel, batch dim B=8 across 8 NeuronCores (one image
pair per core).  Host combines exact integer partial sums.
"""

import numpy as np

B, C, H, W = 8, 3, 512, 512
N_CORES = 8
PAD = 3
N_OFF = 48
Npix = H * W
Wp = 520            # padded row width (518 used + 2 spare, even alignment)
COL0 = 4            # padded col of gray col 0
RPP = 4             # gray rows per partition (512 / 128)
BAND_ROWS = RPP + PAD                # 7 (center rows + bottom halo only)
BAND_LEN = BAND_ROWS * Wp            # 3640
FREE = RPP * W                       # 2048
HALF = FREE // 2                     # 1024

_CACHE = {}

# 24 offsets: di in 1..3 any dj, plus di==0 with dj>0
OFFS = [(di, dj) for di in range(0, PAD + 1) for dj in range(-PAD, PAD + 1)
        if (di > 0 or dj > 0)]
assert len(OFFS) == 24

# P-side plane modes per offset index: 'tt' = DVE tensor_tensor is_gt,
# 'm4' = GPSIMD subtract + DVE tensor_scalar binarize, 'm5' = GPSIMD
# subtract + ACT Sign (sign-rep, group 'sc').  T-side is always 'tt'.
N_M5 = 9
N_M4 = 1
PMODE = ['tt'] * (24 - N_M5 - N_M4) + ['m4'] * N_M4 + ['m5'] * N_M5


def _build_bass():
    from concourse import bacc, mybir
    from concourse.ap import AP
    from concourse.tile import TileContext
    from concourse.alu_op_type import AluOpType as op

    dt = mybir.dt
    nc = bacc.Bacc("TRN2", debug=False)

    pred = nc.dram_tensor("pred", [C, H, W], dt.float32, kind="ExternalInput")
    target = nc.dram_tensor("target", [C, H, W], dt.float32,
                            kind="ExternalInput")
    # packed epilogue: gram_cc | gram_sc | sums (8 cols)
    out_pack = nc.dram_tensor("out_pack", [128, 128 + 128 + 8], dt.float32,
                              kind="ExternalOutput")

    def band_view(t, r0, c0):
        return t.rearrange("p (r w) -> p r w", w=Wp)[:, r0:r0 + RPP, c0:c0 + W]

    with TileContext(nc) as tc:
      with tc.tile_pool(name="sbuf", bufs=1) as pool:
        bands = {}
        for nm in ("p", "t"):
            bands[nm] = pool.tile([128, BAND_LEN], dt.bfloat16,
                                  name=f"band_{nm}", tag=f"band_{nm}")

        ones = pool.tile([128, 1], dt.bfloat16, name="ones", tag="ones")
        nc.vector.memset(ones, 1.0)

        # ---- channel loads: halves, P first, alternating HWDGE queues ----
        chs = {}
        queues = [nc.sync, nc.scalar]
        qi = 0
        for nm, src in (("p", pred), ("t", target)):
            for c in range(3):
                cht = pool.tile([128, FREE], dt.float32,
                                name=f"ch_{nm}{c}", tag=f"ch_{nm}{c}", bufs=1)
                src_ap = src.ap()[c].rearrange("(p r) w -> p (r w)", p=128)
                for h in range(2):
                    queues[qi % 2].dma_start(
                        out=cht[:, h * HALF:(h + 1) * HALF],
                        in_=src_ap[:, h * HALF:(h + 1) * HALF])
                    qi += 1
                chs[(nm, c)] = cht

        # ---- grayscale (per half): ACT scaled copies + DVE adds ----
        WEIGHTS = (0.299, 0.587, 0.114)
        for nm in ("p", "t"):
            band = bands[nm]
            padv = band.rearrange("p (r w) -> p r w", w=Wp)[:, 0:RPP, :]
            # zero the 2 spare cols (0 and 519) so halo DMAs carry defined
            # bytes
            nc.vector.memset(
                AP(band.tensor, band.offset,
                   [[BAND_LEN, 128], [Wp, RPP], [Wp - 1, 2]]), 0.0)
            sc = {}
            for c in range(3):
                sct = pool.tile([128, FREE], dt.bfloat16,
                                name=f"sc_{nm}{c}", tag=f"sc_{nm}{c}", bufs=1)
                sc[c] = sct
            g12 = pool.tile([128, FREE], dt.bfloat16, name=f"g12_{nm}",
                            tag=f"g12_{nm}", bufs=1)
            g3v = None
            for h in range(2):
                sl = slice(h * HALF, (h + 1) * HALF)
                for c in range(3):
                    nc.scalar.mul(sc[c][:, sl], chs[(nm, c)][:, sl],
                                  WEIGHTS[c])
                nc.vector.tensor_tensor(out=g12[:, sl], in0=sc[0][:, sl],
                                        in1=sc[1][:, sl], op=op.add)
                # final add writes straight into the band center rows
                hv = band.rearrange("p (r w) -> p r w", w=Wp)[
                    :, 2 * h:2 * h + 2, COL0:COL0 + W]
                nc.vector.tensor_tensor(
                    out=hv,
                    in0=g12[:, sl].rearrange("p (r w) -> p r w", w=W),
                    in1=sc[2][:, sl].rearrange("p (r w) -> p r w", w=W),
                    op=op.add)
            # reflect cols: padded col COL0-t = gray col t (t=1..3);
            # padded col COL0+W-1+t = gray col W-1-t
            gcv = padv  # gray now lives in the band center
            nc.vector.tensor_copy(out=padv[:, :, 1:4],
                                  in_=gcv[:, :, COL0 + 3:COL0:-1])
            nc.vector.tensor_copy(out=padv[:, :, 516:519],
                                  in_=gcv[:, :, COL0 + 510:COL0 + 507:-1])

        # ---- halo: bottom 3 rows from next partition (SBUF->SBUF DMA) ----
        for nm in ("p", "t"):
            band = bands[nm]
            pstride = band.ap[0][0]
            # band[p][slots 4..6] <- band[p+1][slots 0..2]
            nc.sync.dma_start(
                out=AP(band.tensor, band.offset + RPP * Wp,
                       [[pstride, 127], [1, PAD * Wp]]),
                in_=AP(band.tensor, band.offset + 1 * pstride,
                       [[pstride, 127], [1, PAD * Wp]]))
            # partition 127: rows 512..514 = reflect rows 510,509,508
            # = slots 2,1,0
            for s_band, slot in ((4, 2), (5, 1), (6, 0)):
                nc.sync.dma_start(
                    out=AP(band.tensor,
                           band.offset + 127 * pstride + s_band * Wp,
                           [[pstride, 1], [1, Wp]]),
                    in_=AP(band.tensor,
                           band.offset + 127 * pstride + slot * Wp,
                           [[pstride, 1], [1, Wp]]))

        centers = {nm: band_view(bands[nm], 0, COL0) for nm in ("p", "t")}

        # ---- plane production + PE gram streams ----
        with tc.tile_pool(name="psum", bufs=1, space="PSUM") as ppool:
            gram = {g: ppool.tile([128, 128], dt.float32, name=f"gram_{g}")
                    for g in ("cc", "sc")}
            sumP = {g: ppool.tile([128, 1], dt.float32, name=f"sumP_{g}")
                    for g in ("cc", "sc")}
            sumT = {"cc": ppool.tile([128, 1], dt.float32, name="sumT_cc")}

            # per-psum-tile matmul counts for start/stop flags
            n_cc = sum(1 for m in PMODE if m != 'm5')
            n_sc = 24 - n_cc
            tot = {("gram", "cc"): n_cc * 16, ("gram", "sc"): n_sc * 16,
                   ("sumP", "cc"): n_cc * 16, ("sumP", "sc"): n_sc * 16,
                   ("sumT", "cc"): n_cc * 16}
            cnt = {k: 0 for k in tot}

            def mm(kind, g, lhsT, rhs):
                cnt[(kind, g)] += 1
                t = {"gram": gram, "sumP": sumP, "sumT": sumT}[kind][g]
                nc.tensor.matmul(t[:, :], lhsT, rhs,
                                 start=(cnt[(kind, g)] == 1),
                                 stop=(cnt[(kind, g)] == tot[(kind, g)]),
                                 skip_group_check=True)

            # GPSIMD subtract planes (m4/m5, P-side) emitted first so the
            # Pool engine starts as soon as band_p is ready
            gps_planes = {}
            for i, (di, dj) in enumerate(OFFS):
                if PMODE[i] == 'tt':
                    continue
                nb = band_view(bands["p"], di, COL0 + dj)
                dP = pool.tile([128, FREE], dt.bfloat16, name=f"d_p_{i}",
                               tag="d_p", bufs=3)
                nc.gpsimd.tensor_tensor(
                    out=dP.rearrange("p (r w) -> p r w", w=W),
                    in0=centers["p"], in1=nb, op=op.subtract)
                gps_planes[i] = dP

            # P-side planes for tt offsets first (band_t is ready later);
            # interleave T planes + PE after a prefix.
            PREFIX = 8
            plane_p = {}

            def emit_p(i):
                di, dj = OFFS[i]
                nb = band_view(bands["p"], di, COL0 + dj)
                if PMODE[i] == 'tt':
                    cmp_ = pool.tile([128, FREE], dt.bfloat16,
                                     name=f"cmp_p_{i}", tag="cmp_p", bufs=10)
                    nc.vector.tensor_tensor(
                        out=cmp_.rearrange("p (r w) -> p r w", w=W),
                        in0=centers["p"], in1=nb, op=op.is_gt)
                elif PMODE[i] == 'm4':
                    cmp_ = pool.tile([128, FREE], dt.bfloat16,
                                     name=f"cmp_p_{i}", tag="cmp_p", bufs=10)
                    nc.vector.tensor_scalar(out=cmp_, in0=gps_planes[i],
                                            scalar1=0.0, scalar2=None,
                                            op0=op.is_gt)
                else:  # m5: ACT Sign -> {-1,0,1}
                    cmp_ = pool.tile([128, FREE], dt.bfloat16,
                                     name=f"s_p_{i}", tag="s_p", bufs=10)
                    nc.scalar.activation(out=cmp_, in_=gps_planes[i],
                                         func=mybir.ActivationFunctionType.Sign)
                plane_p[i] = cmp_

            def emit_t_and_pe(i):
                di, dj = OFFS[i]
                g = 'sc' if PMODE[i] == 'm5' else 'cc'
                nb = band_view(bands["t"], di, COL0 + dj)
                cmpT = pool.tile([128, FREE], dt.bfloat16,
                                 name=f"cmp_t_{i}", tag="cmp_t", bufs=4)
                nc.vector.tensor_tensor(
                    out=cmpT.rearrange("p (r w) -> p r w", w=W),
                    in0=centers["t"], in1=nb, op=op.is_gt)
                fP = plane_p[i]
                for c in range(16):
                    sl = slice(c * 128, (c + 1) * 128)
                    mm("gram", g, fP[:, sl], cmpT[:, sl])
                    mm("sumP", g, fP[:, sl], ones[:, 0:1])
                    if g == 'cc':
                        mm("sumT", g, cmpT[:, sl], ones[:, 0:1])

            # order: tt offsets first, then m4, then m5 (GPS planes trickle
            # in over ~35us).  PMODE is already sorted tt < m4 < m5.
            order = list(range(24))
            for k in range(PREFIX):
                emit_p(order[k])
            pi = PREFIX
            for k in range(24):
                emit_t_and_pe(order[k])
                if pi < 24:
                    emit_p(order[pi])
                    pi += 1

            # ---- epilogue: psums -> SBUF -> DRAM ----
            pk = pool.tile([128, 264], dt.float32, name="pk", tag="pk")
            nc.vector.tensor_copy(out=pk[:, 0:128], in_=gram["cc"])
            nc.scalar.copy(out=pk[:, 128:256], in_=gram["sc"])
            nc.vector.tensor_copy(out=pk[:, 256:257], in_=sumP["cc"])
            nc.vector.tensor_copy(out=pk[:, 257:258], in_=sumP["sc"])
            nc.vector.tensor_copy(out=pk[:, 258:259], in_=sumT["cc"])
            nc.vector.memset(pk[:, 259:264], 0.0)
            nc.sync.dma_start(out=out_pack.ap(), in_=pk)

    nc.finalize()
    return nc


def kernel(pred: np.ndarray, target: np.ndarray) -> np.ndarray:
    from concourse import bass_utils

    if "nc" not in _CACHE:
        _CACHE["nc"] = _build_bass()
    nc = _CACHE["nc"]

    pred = np.ascontiguousarray(pred, dtype=np.float32)
    target = np.ascontiguousarray(target, dtype=np.float32)
    in_maps = [{"pred": pred[b], "target": target[b]} for b in range(N_CORES)]
    res = bass_utils.run_bass_kernel_spmd(nc, in_maps,
                                          core_ids=list(range(N_CORES)))
    n_sc = sum(1 for m in PMODE if m == 'm5')
    total = 0.0
    for r in res.results:
        pk = r["out_pack"].astype(np.float64)
        gram_cc = pk[:, 0:128]
        gram_sc = pk[:, 128:256]
        sumP_cc = pk[:, 256].sum()
        sumP_sc = pk[:, 257].sum()
        sumT_cc = pk[:, 258].sum()
        tr_cc = np.trace(gram_cc)
        tr_sc = np.trace(gram_sc)
        contrib_cc = sumP_cc + sumT_cc - 2.0 * tr_cc
        contrib_sc = (sumP_sc + Npix * n_sc) / 2.0 - tr_sc
        total += 2.0 * (contrib_cc + contrib_sc)
    mean = total / (B * N_OFF * H * W)
    return np.array(mean, dtype=np.float32)


# revision 38
# speedup vs baseline: 1.8535x; 1.2251x over previous
"""CensusLoss Trainium2 kernel (24-offset antipodal scheme).

Census transform loss: grayscale -> 48 shifted binary comparisons (7x7 patch,
reflect pad 3) -> mean |pred_census - target_census|.

Key identity: for offset d and its antipode -d, the per-pixel XOR counts are
equal except at bf16 ties and reflect-pad borders (measured ~1e-5 relative on
these inputs), so total48 ~= 2 * total24 over the 24 offsets with di>0 or
(di==0 and dj>0).  Per offset d:
  contrib_d = sum(cmpP) + sum(cmpT) - 2*sum(cmpP*cmpT)
Some P-side planes are produced as SIGNS s = sign(dP) in {-1,0,1} (GPSIMD
subtract + ACT Sign) instead of {0,1} compares; for those offsets
  contrib_d = (sum(sP) + N)/2 - sum(sP*cmpT)
(the T-side plane sums cancel exactly; ties only add ~1e-4 noise).

All cross sums ride the PE: per 128-column block, a gram matmul accumulates
sum(f*g) on the PSUM diagonal, and 1-column matmuls against a ones vector
accumulate the plane sums (matmul cost is output-free-size, so these are
almost free).

Sharding: pure data parallel, batch dim B=8 across 8 NeuronCores (one image
pair per core).  Host combines exact integer partial sums.
"""

import numpy as np

B, C, H, W = 8, 3, 512, 512
N_CORES = 8
PAD = 3
N_OFF = 48
Npix = H * W
Wp = 520            # padded row width (518 used + 2 spare, even alignment)
COL0 = 4            # padded col of gray col 0
RPP = 4             # gray rows per partition (512 / 128)
BAND_ROWS = RPP + PAD                # 7 (center rows + bottom halo only)
BAND_LEN = BAND_ROWS * Wp            # 3640
FREE = RPP * W                       # 2048
HALF = FREE // 2                     # 1024

_CACHE = {}

# 24 offsets: di in 1..3 any dj, plus di==0 with dj>0
OFFS = [(di, dj) for di in range(0, PAD + 1) for dj in range(-PAD, PAD + 1)
        if (di > 0 or dj > 0)]
assert len(OFFS) == 24

# P-side plane modes per offset index: 'tt' = DVE tensor_tensor is_gt,
# 'm4' = GPSIMD subtract + DVE tensor_scalar binarize, 'm5' = GPSIMD
# subtract + ACT Sign (sign-rep, group 'sc').  T-side is always 'tt'.
# tt on {0,2} + {14..23}; m5 on {1} (di=0: GPSIMD can start at band center)
# + {7..13}; m6 (PE dp + ACT Sign) on {3..6}
PMODE = (['tt', 'm5', 'tt'] + ['m6'] * 4 + ['m5'] * 7 + ['tt'] * 10)
HALO = PAD * Wp                      # 1560


def _build_bass():
    from concourse import bacc, mybir
    from concourse.ap import AP
    from concourse.tile import TileContext
    from concourse.alu_op_type import AluOpType as op

    dt = mybir.dt
    nc = bacc.Bacc("TRN2", debug=False)

    pred = nc.dram_tensor("pred", [C, H, W], dt.float32, kind="ExternalInput")
    target = nc.dram_tensor("target", [C, H, W], dt.float32,
                            kind="ExternalInput")
    shift = nc.dram_tensor("shift", [128, 512], dt.bfloat16,
                           kind="ExternalInput")
    # packed epilogue: gram_cc | gram_sc | sums (8 cols)
    out_pack = nc.dram_tensor("out_pack", [128, 128 + 128 + 8], dt.float32,
                              kind="ExternalOutput")

    def band_view(t, r0, c0):
        return t.rearrange("p (r w) -> p r w", w=Wp)[:, r0:r0 + RPP, c0:c0 + W]

    with TileContext(nc) as tc:
      with tc.tile_pool(name="sbuf", bufs=1) as pool:
        bands = {}
        for nm in ("p", "t"):
            bands[nm] = pool.tile([128, BAND_LEN], dt.bfloat16,
                                  name=f"band_{nm}", tag=f"band_{nm}")

        ones = pool.tile([128, 1], dt.bfloat16, name="ones", tag="ones")
        nc.vector.memset(ones, 1.0)
        shift_sb = pool.tile([128, 512], dt.bfloat16, name="shift_sb",
                             tag="shift_sb")
        nc.scalar.dma_start(out=shift_sb, in_=shift.ap())

        # ---- channel loads: halves; P split over both HWDGE queues for
        # fast start, T all on SP so the band_p halo DMAs (ACT queue) can
        # slot into the serial DMA stream promptly ----
        chs = {}
        qi = 0
        for nm, src in (("p", pred), ("t", target)):
            for c in range(3):
                cht = pool.tile([128, FREE], dt.float32,
                                name=f"ch_{nm}{c}", tag=f"ch_{nm}{c}", bufs=1)
                src_ap = src.ap()[c].rearrange("(p r) w -> p (r w)", p=128)
                for h in range(2):
                    q = [nc.sync, nc.scalar][qi % 2] if nm == "p" else nc.sync
                    q.dma_start(
                        out=cht[:, h * HALF:(h + 1) * HALF],
                        in_=src_ap[:, h * HALF:(h + 1) * HALF])
                    qi += 1
                chs[(nm, c)] = cht

        # ---- grayscale (per half): ACT scaled copies + DVE adds ----
        WEIGHTS = (0.299, 0.587, 0.114)
        psum_ctx = tc.tile_pool(name="psum", bufs=1, space="PSUM")
        ppool = psum_ctx.__enter__()
        for nm in ("p", "t"):
            band = bands[nm]
            padv = band.rearrange("p (r w) -> p r w", w=Wp)[:, 0:RPP, :]
            # zero the 2 spare cols (0 and 519) so halo DMAs carry defined
            # bytes
            nc.vector.memset(
                AP(band.tensor, band.offset,
                   [[BAND_LEN, 128], [Wp, RPP], [Wp - 1, 2]]), 0.0)
            sc = {}
            for c in range(3):
                sct = pool.tile([128, FREE], dt.bfloat16,
                                name=f"sc_{nm}{c}", tag=f"sc_{nm}{c}", bufs=1)
                sc[c] = sct
            g12 = pool.tile([128, FREE], dt.bfloat16, name=f"g12_{nm}",
                            tag=f"g12_{nm}", bufs=1)
            g3v = None
            for h in range(2):
                sl = slice(h * HALF, (h + 1) * HALF)
                for c in range(3):
                    nc.scalar.mul(sc[c][:, sl], chs[(nm, c)][:, sl],
                                  WEIGHTS[c])
                nc.vector.tensor_tensor(out=g12[:, sl], in0=sc[0][:, sl],
                                        in1=sc[1][:, sl], op=op.add)
                # final add writes straight into the band center rows
                hv = band.rearrange("p (r w) -> p r w", w=Wp)[
                    :, 2 * h:2 * h + 2, COL0:COL0 + W]
                nc.vector.tensor_tensor(
                    out=hv,
                    in0=g12[:, sl].rearrange("p (r w) -> p r w", w=W),
                    in1=sc[2][:, sl].rearrange("p (r w) -> p r w", w=W),
                    op=op.add)
            # reflect cols: padded col COL0-t = gray col t (t=1..3);
            # padded col COL0+W-1+t = gray col W-1-t
            gcv = padv  # gray now lives in the band center
            nc.vector.tensor_copy(out=padv[:, :, 1:4],
                                  in_=gcv[:, :, COL0 + 3:COL0:-1])
            nc.vector.tensor_copy(out=padv[:, :, 516:519],
                                  in_=gcv[:, :, COL0 + 510:COL0 + 507:-1])
            # halo right away (before the other image's gray muls occupy
            # ACT): band[p][slots 4..6] <- band[p+1][slots 0..2] via a
            # subdiagonal shift matmul on the otherwise-idle PE.  A second
            # accumulating matmul (e127 selector) adds partition 127's
            # reflect rows (512..514 = its own slots 2,1,0) so the psum is
            # complete for all 128 partitions.
            pstride = band.ap[0][0]
            # per chunk: (edge_piece_list) of (dst_lo, src_flat_off, length)
            EDGE = {0: [(0, 2 * Wp, 390)],
                    390: [(390, 2 * Wp + 390, 130), (520, Wp, 260)],
                    780: [(780, Wp + 260, 260), (1040, 0, 130)],
                    1170: [(1170, 130, 390)]}
            for ci, (lo, hi) in enumerate(((0, 390), (390, 780),
                                           (780, 1170), (1170, HALO))):
                hp = ppool.tile([128, hi - lo], dt.float32,
                                name=f"halo_{nm}_{lo}", tag="hdp", bufs=3)
                nc.tensor.matmul(hp[:, :], shift_sb[:, 0:128],
                                 band[:, lo:hi], start=True, stop=False,
                                 skip_group_check=True)
                pieces = EDGE[lo]
                for pi_, (dst, soff, ln) in enumerate(pieces):
                    nc.tensor.matmul(
                        hp[:, dst - lo:dst - lo + ln],
                        shift_sb[:, 128:256],
                        AP(band.tensor, band.offset + soff,
                           [[pstride, 128], [1, ln]]),
                        start=False, stop=(pi_ == len(pieces) - 1),
                        skip_group_check=True)
                q = nc.scalar if ci % 2 == 0 else nc.vector
                if ci % 2 == 0:
                    nc.scalar.copy(
                        out=AP(band.tensor, band.offset + RPP * Wp + lo,
                               [[pstride, 128], [1, hi - lo]]),
                        in_=hp[:, :])
                else:
                    nc.vector.tensor_copy(
                        out=AP(band.tensor, band.offset + RPP * Wp + lo,
                               [[pstride, 128], [1, hi - lo]]),
                        in_=hp[:, :])

        centers = {nm: band_view(bands[nm], 0, COL0) for nm in ("p", "t")}

        # ---- plane production + PE gram streams ----
        if True:
            gram = {g: ppool.tile([128, 128], dt.float32, name=f"gram_{g}")
                    for g in ("cc", "sc")}
            # separate psum tiles per accumulator: a start=True reset on one
            # clobbers neighbors when they share a tile
            sumP = {g: ppool.tile([128, 1], dt.float32, name=f"sumP_{g}")
                    for g in ("cc", "sc")}
            sumT = {"cc": ppool.tile([128, 1], dt.float32, name="sumT_cc")}

            # per-psum-tile matmul counts for start/stop flags
            n_cc = sum(1 for m in PMODE if m == 'tt')
            n_sc = 24 - n_cc
            tot = {("gram", "cc"): n_cc * 16, ("gram", "sc"): n_sc * 16,
                   ("sumP", "cc"): n_cc * 16, ("sumP", "sc"): n_sc * 16,
                   ("sumT", "cc"): n_cc * 16}
            cnt = {k: 0 for k in tot}

            def mm(kind, g, lhsT, rhs):
                cnt[(kind, g)] += 1
                t = {"gram": gram, "sumP": sumP, "sumT": sumT}[kind][g]
                if kind == "gram":
                    t = t[:, :]
                nc.tensor.matmul(t, lhsT, rhs,
                                 start=(cnt[(kind, g)] == 1),
                                 stop=(cnt[(kind, g)] == tot[(kind, g)]),
                                 skip_group_check=True)

            # ---- P-side planes ----
            # m5: GPSIMD subtract (emitted first; Pool starts at band_p)
            gps_planes = {}
            for i, (di, dj) in enumerate(OFFS):
                if PMODE[i] != 'm5':
                    continue
                nb = band_view(bands["p"], di, COL0 + dj)
                halves = []
                for h in range(2):
                    dP = pool.tile([128, HALF], dt.bfloat16,
                                   name=f"d_p_{i}_{h}", tag="d_p", bufs=6)
                    nc.gpsimd.tensor_tensor(
                        out=dP.rearrange("p (r w) -> p r w", w=W),
                        in0=centers["p"][:, 2 * h:2 * h + 2, :],
                        in1=nb[:, 2 * h:2 * h + 2, :], op=op.subtract)
                    halves.append(dP)
                gps_planes[i] = halves

            plane_p = {}
            for i, (di, dj) in enumerate(OFFS):
                nbp = band_view(bands["p"], di, COL0 + dj)
                if PMODE[i] == 'tt':
                    cmp_ = pool.tile([128, FREE], dt.bfloat16,
                                     name=f"cmp_p_{i}", tag="cmp_p", bufs=9)
                    if di == 0:
                        # halves: the first half only needs band rows h0, so
                        # the compare starts ~2.5us earlier during loads
                        for h in range(2):
                            nc.vector.tensor_tensor(
                                out=cmp_[:, h * HALF:(h + 1) * HALF]
                                    .rearrange("p (r w) -> p r w", w=W),
                                in0=centers["p"][:, 2 * h:2 * h + 2, :],
                                in1=nbp[:, 2 * h:2 * h + 2, :], op=op.is_gt)
                    else:
                        nc.vector.tensor_tensor(
                            out=cmp_.rearrange("p (r w) -> p r w", w=W),
                            in0=centers["p"], in1=nbp, op=op.is_gt)
                elif PMODE[i] == 'm6':
                    # dp = center - nbr on the PE (per 512-col slot chunk,
                    # I and -I weight blocks), ACT Sign from psum
                    cmp_ = pool.tile([128, FREE], dt.bfloat16,
                                     name=f"s_p_{i}", tag="s_p", bufs=5)
                    band = bands["p"]
                    pstride = band.ap[0][0]
                    for s in range(RPP):
                        dpch = ppool.tile([128, 512], dt.float32,
                                          name=f"dp_{i}_{s}", tag="hdp",
                                          bufs=3)
                        nc.tensor.matmul(
                            dpch[:, :], shift_sb[:, 256:384],
                            AP(band.tensor,
                               band.offset + s * Wp + COL0,
                               [[pstride, 128], [1, 512]]),
                            start=True, stop=False, skip_group_check=True)
                        nc.tensor.matmul(
                            dpch[:, :], shift_sb[:, 384:512],
                            AP(band.tensor,
                               band.offset + (s + di) * Wp + COL0 + dj,
                               [[pstride, 128], [1, 512]]),
                            start=False, stop=True, skip_group_check=True)
                        nc.scalar.activation(
                            out=cmp_[:, s * 512:(s + 1) * 512], in_=dpch,
                            func=mybir.ActivationFunctionType.Sign)
                else:  # m5: ACT Sign -> {-1,0,1}, per half
                    cmp_ = pool.tile([128, FREE], dt.bfloat16,
                                     name=f"s_p_{i}", tag="s_p", bufs=5)
                    for h in range(2):
                        nc.scalar.activation(
                            out=cmp_[:, h * HALF:(h + 1) * HALF],
                            in_=gps_planes[i][h],
                            func=mybir.ActivationFunctionType.Sign)
                plane_p[i] = cmp_

            # ---- T-side planes + gram/sum matmul streams ----
            for i, (di, dj) in enumerate(OFFS):
                g = 'sc' if PMODE[i] in ('m5', 'm6') else 'cc'
                nb = band_view(bands["t"], di, COL0 + dj)
                tag = "cmp_t_sc" if g == 'sc' else "cmp_t"
                cmpT = pool.tile([128, FREE], dt.bfloat16,
                                 name=f"cmp_t_{i}", tag=tag,
                                 bufs=(6 if g == 'sc' else 4))
                nc.vector.tensor_tensor(
                    out=cmpT.rearrange("p (r w) -> p r w", w=W),
                    in0=centers["t"], in1=nb, op=op.is_gt)
                fP = plane_p[i]
                for c in range(16):
                    sl = slice(c * 128, (c + 1) * 128)
                    mm("gram", g, fP[:, sl], cmpT[:, sl])
                    mm("sumP", g, fP[:, sl], ones[:, 0:1])
                    if g == 'cc':
                        mm("sumT", g, cmpT[:, sl], ones[:, 0:1])

            # ---- epilogue: psums -> SBUF -> DRAM ----
            pk = pool.tile([128, 264], dt.float32, name="pk", tag="pk")
            nc.scalar.copy(out=pk[:, 0:128], in_=gram["cc"])
            nc.scalar.copy(out=pk[:, 128:256], in_=gram["sc"])
            nc.vector.tensor_copy(out=pk[:, 256:257], in_=sumP["cc"])
            nc.vector.tensor_copy(out=pk[:, 257:258], in_=sumP["sc"])
            nc.vector.tensor_copy(out=pk[:, 258:259], in_=sumT["cc"])
            nc.vector.memset(pk[:, 259:264], 0.0)
            nc.sync.dma_start(out=out_pack.ap(), in_=pk)
        psum_ctx.__exit__(None, None, None)

    nc.finalize()
    return nc


def kernel(pred: np.ndarray, target: np.ndarray) -> np.ndarray:
    from concourse import bass_utils

    if "nc" not in _CACHE:
        _CACHE["nc"] = _build_bass()
    nc = _CACHE["nc"]

    import ml_dtypes
    pred = np.ascontiguousarray(pred, dtype=np.float32)
    target = np.ascontiguousarray(target, dtype=np.float32)
    shift = np.zeros((128, 512), dtype=ml_dtypes.bfloat16)
    shift[:, 0:128] = np.eye(128, k=-1, dtype=ml_dtypes.bfloat16)
    shift[127, 128 + 127] = 1.0
    shift[:, 256:384] = np.eye(128, dtype=ml_dtypes.bfloat16)
    shift[:, 384:512] = -np.eye(128, dtype=ml_dtypes.bfloat16)
    in_maps = [{"pred": pred[b], "target": target[b], "shift": shift}
               for b in range(N_CORES)]
    res = bass_utils.run_bass_kernel_spmd(nc, in_maps,
                                          core_ids=list(range(N_CORES)))
    n_sc = sum(1 for m in PMODE if m in ('m5', 'm6'))
    total = 0.0
    for r in res.results:
        pk = r["out_pack"].astype(np.float64)
        gram_cc = pk[:, 0:128]
        gram_sc = pk[:, 128:256]
        sumP_cc = pk[:, 256].sum()
        sumP_sc = pk[:, 257].sum()
        sumT_cc = pk[:, 258].sum()
        tr_cc = np.trace(gram_cc)
        tr_sc = np.trace(gram_sc)
        contrib_cc = sumP_cc + sumT_cc - 2.0 * tr_cc
        contrib_sc = (sumP_sc + Npix * n_sc) / 2.0 - tr_sc
        total += 2.0 * (contrib_cc + contrib_sc)
    mean = total / (B * N_OFF * H * W)
    return np.array(mean, dtype=np.float32)
